# revision 27
# baseline (speedup 1.0000x reference)
"""Trainium2 Bass kernel: cluster-memory cross-entropy loss.

Computes loss = mean_b [ logsumexp_n((x_b . f_n)/T) - (x_b . f_{t_b})/T ]
with x = L2-normalized inputs [B,D], f = memory bank [N,D] (pre-normalized),
T = 0.05, B=4096, N=131072, D=256.

Strategy (vocab-parallel over 8 NeuronCores):
 - Shard features along N: 16384 rows per core.
 - Host: normalize x (fp32), cast x & features to bf16, pre-transpose to
   K-major layouts for the PE.
 - Device (per core): stream the [4096 x 16384] logit block through the
   tensor engine in [128 x 2048] PSUM groups (bf16 matmul, fp32 accum,
   K=256 in 2 chunks); ScalarE applies exp(20*cos - 20) with the fused
   affine (scale=20, bias=-20; cos<=1 so the fixed shift of 20 is an
   upper bound of every logit) writing bf16 to SBUF; VectorE (otherwise
   idle) does the per-partition row sums via tensor_scalar accum_out at
   its 4x bf16 rate. Only the [128, 256] per-(row, group) partial sums
   leave the chip. ScalarE's 1 elem/lane/cycle exp throughput is the
   bound: 67.1M exps / (128 lanes * 1.2GHz) = 437us; everything else
   hides under it.
 - Host: sum partials over groups/cores (fp64), lse = 20 + log(S),
   target logits computed exactly on host (tiny gather+dot), mean.
"""

import sys

for _p in ("/opt/trn_rl_repo", "/root/.axon_site/_ro/trn_rl_repo"):
    if _p not in sys.path:
        sys.path.append(_p)

import numpy as np
import ml_dtypes

P = 128
B = 4096
N = 131072
D = 256
TEMP = 0.05
EPS = 1e-12
NCORES = 8
NSH = N // NCORES          # 16384 classes per core
KC = D // P                # 2 contraction chunks
M_TILES = B // P           # 32 row tiles
GW = 2048                  # PSUM group width (4 banks)
NG = NSH // GW             # 8 groups per row tile
MM_N = 512                 # matmul free dim (1 PSUM bank)
JPG = GW // MM_N           # 4 matmuls per group per k
SHIFT = 1.0 / TEMP         # 20.0; logits = SHIFT * cos, max logit <= SHIFT
FP8_SCALE = 16.0           # pre-scale for e4m3 inputs (entries ~N(0,1/256))

_CACHE = {}
SCRATCH_BUFS = 2
PSUM_BUFS = 2


def _build_module(loop_reps=1, variant="dve_ts_accum"):
    """loop_reps > 1 wraps the compute in a runtime For_i that recomputes the
    identical result; used only to amplify execution time for benchmarking.

    variants (measured on HW via the For_i delta method):
      dve_ts_accum      - ACT plain exp -> bf16 SBUF; DVE tensor_scalar w/
                          accum_out does row sums (4x bf16). 512us. DEFAULT.
      act_accum         - ACT exp with fused accum_out; pays a ~90ns/instr
                          accumulator read on the serial ACT engine. 535us.
      act_accum_inplace - act_accum with exp written in place to PSUM.
      dve_reduce        - DVE tensor_reduce for row sums (1x; DVE-bound).
      fp8_dr            - dve_ts_accum with e4m3 DoubleRow matmuls (K=256 in
                          one MM). Same 512us on HW (ACT-bound either way)
                          but ~400x worse loss error; kept for reference.
    """
    import contextlib

    import concourse.tile as tile
    import concourse.mybir as mybir
    from concourse import bacc

    nc = bacc.Bacc("TRN2", target_bir_lowering=False)
    bf16 = mybir.dt.bfloat16
    f32 = mybir.dt.float32
    fp8 = variant == "fp8_dr"
    in_dt = mybir.dt.float8e4 if fp8 else bf16
    # fp8 inputs are pre-scaled by 16 on the host, so PSUM holds 256*cos;
    # fold the 1/256 into the ACT affine.
    act_scale = SHIFT / (FP8_SCALE * FP8_SCALE) if fp8 else SHIFT

    # fp8 uses partition-major [P, KC, *] DRAM layouts matching the DoubleRow
    # [partition, k-pair, free] SBUF tiles.
    if fp8:
        xT = nc.dram_tensor("xT", [P, KC, B], in_dt, kind="ExternalInput")
        fT = nc.dram_tensor("fT", [P, KC, NSH], in_dt, kind="ExternalInput")
    else:
        xT = nc.dram_tensor("xT", [KC, P, B], in_dt, kind="ExternalInput")
        fT = nc.dram_tensor("fT", [KC, P, NSH], in_dt, kind="ExternalInput")
    s_out = nc.dram_tensor("s_out", [P, M_TILES * NG], f32, kind="ExternalOutput")

    with tile.TileContext(nc) as tc:
        with (
            tc.tile_pool(name="resident", bufs=1) as resident,
            tc.tile_pool(name="psum", bufs=PSUM_BUFS, space="PSUM") as psum_pool,
            tc.tile_pool(name="scratch", bufs=SCRATCH_BUFS) as scratch,
        ):
            bias_t = resident.tile([P, 1], f32, name="bias_t")
            nc.vector.memset(bias_t[:], -SHIFT)

            acc_sb = resident.tile([P, M_TILES * NG], f32, name="acc_sb")

            # x split into per-m-tile pieces so the first matmul group only
            # waits for a sliver of x instead of the full tensor. DMA order:
            # f group 0 first (first compute group), then x pieces, then the
            # remaining f groups.
            if fp8:
                # DoubleRow layout: [partition, k-pair, free] in one tile so a
                # single matmul contracts all of K=256.
                x_sb = [
                    resident.tile([P, KC, P], in_dt, name=f"x8_{m}")
                    for m in range(M_TILES)
                ]
                f_sb = [
                    resident.tile([P, KC, GW], in_dt, name=f"f8_{g}")
                    for g in range(NG)
                ]
                nc.sync.dma_start(f_sb[0][:], fT[:, :, 0:GW])
                for m in range(M_TILES):
                    nc.sync.dma_start(x_sb[m][:], xT[:, :, m * P : (m + 1) * P])
                for g in range(1, NG):
                    nc.sync.dma_start(f_sb[g][:], fT[:, :, g * GW : (g + 1) * GW])
            else:
                x_sb = [
                    [
                        resident.tile([P, P], bf16, name=f"x_sb_{k}_{m}")
                        for m in range(M_TILES)
                    ]
                    for k in range(KC)
                ]
                f_sb = [
                    [
                        resident.tile([P, GW], bf16, name=f"f_sb_{k}_{g}")
                        for g in range(NG)
                    ]
                    for k in range(KC)
                ]
                for k in range(KC):
                    nc.sync.dma_start(f_sb[k][0][:], fT[k, :, 0:GW])
                for m in range(M_TILES):
                    for k in range(KC):
                        nc.sync.dma_start(
                            x_sb[k][m][:], xT[k, :, m * P : (m + 1) * P]
                        )
                for g in range(1, NG):
                    for k in range(KC):
                        nc.sync.dma_start(
                            f_sb[k][g][:], fT[k, :, g * GW : (g + 1) * GW]
                        )

            loop_cm = (
                tc.For_i(0, loop_reps, 1)
                if loop_reps > 1
                else contextlib.nullcontext()
            )
            with loop_cm:
                # g outer / m inner: once f group g lands in SBUF there are 32
                # row-tiles (~60us) of work against it, so compute never
                # starves on the f DMA stream.
                for g in range(NG):
                    for m in range(M_TILES):
                        pt = psum_pool.tile([P, GW], f32, name="pt")
                        if fp8:
                            for j in range(JPG):
                                nc.tensor.matmul(
                                    pt[:, j * MM_N : (j + 1) * MM_N],
                                    lhsT=x_sb[m][:],
                                    rhs=f_sb[g][:, :, j * MM_N : (j + 1) * MM_N],
                                    start=True,
                                    stop=True,
                                    perf_mode=mybir.MatmulPerfMode.DoubleRow,
                                )
                        else:
                            for k in range(KC):
                                for j in range(JPG):
                                    nc.tensor.matmul(
                                        pt[:, j * MM_N : (j + 1) * MM_N],
                                        lhsT=x_sb[k][m][:],
                                        rhs=f_sb[k][g][:, j * MM_N : (j + 1) * MM_N],
                                        start=(k == 0),
                                        stop=(k == KC - 1),
                                    )
                        col = m * NG + g
                        if variant == "act_accum":
                            ex = scratch.tile([P, GW], bf16, name="ex")
                            nc.scalar.activation(
                                ex[:],
                                pt[:],
                                mybir.ActivationFunctionType.Exp,
                                bias=bias_t[:, 0:1],
                                scale=act_scale,
                                accum_out=acc_sb[:, col : col + 1],
                            )
                        elif variant == "act_accum_inplace":
                            nc.scalar.activation(
                                pt[:],
                                pt[:],
                                mybir.ActivationFunctionType.Exp,
                                bias=bias_t[:, 0:1],
                                scale=act_scale,
                                accum_out=acc_sb[:, col : col + 1],
                            )
                        elif variant in ("dve_ts_accum", "fp8_dr"):
                            ex = scratch.tile([P, GW], bf16, name="ex")
                            junk = scratch.tile([P, GW], bf16, name="junk")
                            nc.scalar.activation(
                                ex[:],
                                pt[:],
                                mybir.ActivationFunctionType.Exp,
                                bias=bias_t[:, 0:1],
                                scale=act_scale,
                            )
                            nc.vector.tensor_scalar(
                                junk[:],
                                ex[:],
                                1.0,
                                0.0,
                                mybir.AluOpType.mult,
                                mybir.AluOpType.add,
                                accum_out=acc_sb[:, col : col + 1],
                            )
                        elif variant == "dve_reduce":
                            ex = scratch.tile([P, GW], bf16, name="ex")
                            nc.scalar.activation(
                                ex[:],
                                pt[:],
                                mybir.ActivationFunctionType.Exp,
                                bias=bias_t[:, 0:1],
                                scale=act_scale,
                            )
                            nc.vector.tensor_reduce(
                                acc_sb[:, col : col + 1],
                                ex[:],
                                axis=mybir.AxisListType.X,
                                op=mybir.AluOpType.add,
                            )
                        else:
                            raise ValueError(variant)

            nc.sync.dma_start(s_out[:, :], acc_sb[:])

    nc.compile()
    return nc


VARIANT = "dve_ts_accum"  # fp8_dr measured identical on HW; bf16 is more accurate


def _prep_in_maps(x, features, variant):
    if variant == "fp8_dr":
        xq = (x * FP8_SCALE).astype(ml_dtypes.float8_e4m3)
        xT = np.ascontiguousarray(xq.T.reshape(KC, P, B).transpose(1, 0, 2))
        fq = (features * FP8_SCALE).astype(ml_dtypes.float8_e4m3)
        in_maps = []
        for c in range(NCORES):
            shard = fq[c * NSH : (c + 1) * NSH]
            fT = np.ascontiguousarray(
                shard.T.reshape(KC, P, NSH).transpose(1, 0, 2)
            )
            in_maps.append({"xT": xT, "fT": fT})
    else:
        xb = x.astype(ml_dtypes.bfloat16)
        xT = np.ascontiguousarray(xb.T).reshape(KC, P, B)
        fb = features.astype(ml_dtypes.bfloat16)
        in_maps = []
        for c in range(NCORES):
            shard = fb[c * NSH : (c + 1) * NSH]      # [NSH, D]
            fT = np.ascontiguousarray(shard.T).reshape(KC, P, NSH)
            in_maps.append({"xT": xT, "fT": fT})
    return in_maps


def kernel(inputs, targets, feature_memory, k, features):
    from concourse.bass_utils import run_bass_kernel_spmd

    inputs = np.asarray(inputs, dtype=np.float32)
    targets = np.asarray(targets).astype(np.int64)
    features = np.asarray(features, dtype=np.float32)

    # Host prep: normalize (fp32, matching the reference), quantize,
    # pre-transpose to the PE-friendly layouts.
    nrm = np.linalg.norm(inputs, axis=1, keepdims=True)
    x = inputs / np.maximum(nrm, EPS)
    in_maps = _prep_in_maps(x, features, VARIANT)

    # Exact target logits on host (tiny: B dot-products).
    f_t = features[targets].astype(np.float64)
    tgt = SHIFT * np.einsum("bd,bd->b", x.astype(np.float64), f_t)

    if "nc" not in _CACHE:
        _CACHE["nc"] = _build_module(variant=VARIANT)
    nc = _CACHE["nc"]

    res = run_bass_kernel_spmd(nc, in_maps, core_ids=list(range(NCORES)))

    # Unshard: S_total[b] = sum over cores/groups; b = m*128 + p.
    S = np.zeros(B, dtype=np.float64)
    for c in range(NCORES):
        R = res.results[c]["s_out"].astype(np.float64)        # [P, M_TILES*NG]
        S += R.reshape(P, M_TILES, NG).sum(axis=2).T.reshape(B)

    lse = SHIFT + np.log(S)
    loss = np.mean(lse - tgt)
    return np.array(loss, dtype=np.float32)


# revision 30
# speedup vs baseline: 1.5864x; 1.5864x over previous
"""Trainium2 Bass kernel: cluster-memory cross-entropy loss.

Computes loss = mean_b [ logsumexp_n((x_b . f_n)/T) - (x_b . f_{t_b})/T ]
with x = L2-normalized inputs [B,D], f = memory bank [N,D] (pre-normalized),
T = 0.05, B=4096, N=131072, D=256.

Strategy (vocab-parallel over 8 NeuronCores):
 - Shard features along N: 16384 rows per core.
 - Host: normalize x (fp32), cast x & features to bf16, pre-transpose to
   K-major layouts for the PE.
 - Device (per core): stream the [4096 x 16384] logit block through the
   tensor engine in [128 x 2048] PSUM groups (bf16 matmul, fp32 accum,
   K=256 in 2 chunks); ScalarE applies exp(20*cos - 20) with the fused
   affine (scale=20, bias=-20; cos<=1 so the fixed shift of 20 is an
   upper bound of every logit) writing bf16 to SBUF; VectorE (otherwise
   idle) does the per-partition row sums via tensor_scalar accum_out at
   its 4x bf16 rate. Only the [128, 256] per-(row, group) partial sums
   leave the chip. ScalarE's 1 elem/lane/cycle exp throughput is the
   bound: 67.1M exps / (128 lanes * 1.2GHz) = 437us; everything else
   hides under it.
 - Host: sum partials over groups/cores (fp64), lse = 20 + log(S),
   target logits computed exactly on host (tiny gather+dot), mean.
"""

import sys

for _p in ("/opt/trn_rl_repo", "/root/.axon_site/_ro/trn_rl_repo"):
    if _p not in sys.path:
        sys.path.append(_p)

import numpy as np
import ml_dtypes

P = 128
B = 4096
N = 131072
D = 256
TEMP = 0.05
EPS = 1e-12
NCORES = 8
NSH = N // NCORES          # 16384 classes per core
KC = D // P                # 2 contraction chunks
M_TILES = B // P           # 32 row tiles
FW = 2048                  # f-tile width in SBUF (DMA granularity)
PW = 2048                  # PSUM group width (PW*4B/2KB banks per group)
NG = NSH // PW             # accumulation groups per row tile
MM_N = 512                 # matmul free dim (1 PSUM bank)
JPG = PW // MM_N           # matmuls per group per k
SHIFT = 1.0 / TEMP         # 20.0; logits = SHIFT * cos, max logit <= SHIFT
FP8_SCALE = 16.0           # pre-scale for e4m3 inputs (entries ~N(0,1/256))

_CACHE = {}
SCRATCH_BUFS = 2
PSUM_BUFS = 2


def _build_module(loop_reps=1, variant="dve_ts_accum"):
    """loop_reps > 1 wraps the compute in a runtime For_i that recomputes the
    identical result; used only to amplify execution time for benchmarking.

    variants (measured on HW via the For_i delta method):
      dve_ts_accum      - ACT plain exp -> bf16 SBUF; DVE tensor_scalar w/
                          accum_out does row sums (4x bf16). 512us. DEFAULT.
      act_accum         - ACT exp with fused accum_out; pays a ~90ns/instr
                          accumulator read on the serial ACT engine. 535us.
      act_accum_inplace - act_accum with exp written in place to PSUM.
      dve_reduce        - DVE tensor_reduce for row sums (1x; DVE-bound).
      fp8_dr            - dve_ts_accum with e4m3 DoubleRow matmuls (K=256 in
                          one MM). Same 512us on HW (ACT-bound either way)
                          but ~400x worse loss error; kept for reference.
    """
    import contextlib

    import concourse.tile as tile
    import concourse.mybir as mybir
    from concourse import bacc

    nc = bacc.Bacc("TRN2", target_bir_lowering=False)
    bf16 = mybir.dt.bfloat16
    f32 = mybir.dt.float32
    fp8 = variant == "fp8_dr"
    in_dt = mybir.dt.float8e4 if fp8 else bf16
    # fp8 inputs are pre-scaled by 16 on the host, so PSUM holds 256*cos;
    # fold the 1/256 into the ACT affine.
    act_scale = SHIFT / (FP8_SCALE * FP8_SCALE) if fp8 else SHIFT

    # fp8 uses partition-major [P, KC, *] DRAM layouts matching the DoubleRow
    # [partition, k-pair, free] SBUF tiles.
    if fp8:
        xT = nc.dram_tensor("xT", [P, KC, B], in_dt, kind="ExternalInput")
        fT = nc.dram_tensor("fT", [P, KC, NSH], in_dt, kind="ExternalInput")
    else:
        xT = nc.dram_tensor("xT", [KC, P, B], in_dt, kind="ExternalInput")
        fT = nc.dram_tensor("fT", [KC, P, NSH], in_dt, kind="ExternalInput")
    s_out = nc.dram_tensor("s_out", [P, M_TILES * NG], f32, kind="ExternalOutput")

    with tile.TileContext(nc) as tc:
        with (
            tc.tile_pool(name="resident", bufs=1) as resident,
            tc.tile_pool(name="psum", bufs=PSUM_BUFS, space="PSUM") as psum_pool,
            tc.tile_pool(name="scratch", bufs=SCRATCH_BUFS) as scratch,
        ):
            bias_t = resident.tile([P, 1], f32, name="bias_t")
            nc.vector.memset(bias_t[:], -SHIFT)

            acc_sb = resident.tile([P, M_TILES * NG], f32, name="acc_sb")

            # x split into per-m-tile pieces so the first matmul group only
            # waits for a sliver of x instead of the full tensor. DMA order:
            # f group 0 first (first compute group), then x pieces, then the
            # remaining f groups.
            if fp8:
                # DoubleRow layout: [partition, k-pair, free] in one tile so a
                # single matmul contracts all of K=256.
                x_sb = [
                    resident.tile([P, KC, P], in_dt, name=f"x8_{m}")
                    for m in range(M_TILES)
                ]
                f_sb = [
                    resident.tile([P, KC, FW], in_dt, name=f"f8_{g}")
                    for g in range(NSH // FW)
                ]
                nc.sync.dma_start(f_sb[0][:], fT[:, :, 0:FW])
                for m in range(M_TILES):
                    nc.sync.dma_start(x_sb[m][:], xT[:, :, m * P : (m + 1) * P])
                for g in range(1, NSH // FW):
                    nc.sync.dma_start(f_sb[g][:], fT[:, :, g * FW : (g + 1) * FW])
            else:
                x_sb = [
                    [
                        resident.tile([P, P], bf16, name=f"x_sb_{k}_{m}")
                        for m in range(M_TILES)
                    ]
                    for k in range(KC)
                ]
                f_sb = [
                    [
                        resident.tile([P, FW], bf16, name=f"f_sb_{k}_{g}")
                        for g in range(NSH // FW)
                    ]
                    for k in range(KC)
                ]
                for k in range(KC):
                    nc.sync.dma_start(f_sb[k][0][:], fT[k, :, 0:FW])
                for m in range(M_TILES):
                    for k in range(KC):
                        nc.sync.dma_start(
                            x_sb[k][m][:], xT[k, :, m * P : (m + 1) * P]
                        )
                for g in range(1, NSH // FW):
                    for k in range(KC):
                        nc.sync.dma_start(
                            f_sb[k][g][:], fT[k, :, g * FW : (g + 1) * FW]
                        )

            # Phantom warm-up group: dummy matmuls on memset junk keep the PE
            # busy during the initial DMA so the HAM clock-gate releases
            # (1.2 -> 2.4 GHz) before the first real group, and the exp table
            # load isn't the first thing ACT does. Uses one rotation of the
            # psum pool; nothing reads the results.
            warm_w = scratch.tile([P, P], bf16, name="warm_w")
            warm_r = scratch.tile([P, MM_N], bf16, name="warm_r")
            nc.vector.memset(warm_w[:], 0.0)
            nc.vector.memset(warm_r[:], 0.0)
            warm_pt = psum_pool.tile([P, PW], f32, name="pt")
            for j in range(JPG * KC):
                nc.tensor.matmul(
                    warm_pt[:, (j % JPG) * MM_N : (j % JPG + 1) * MM_N],
                    lhsT=warm_w[:],
                    rhs=warm_r[:],
                    start=(j < JPG),
                    stop=(j >= JPG),
                )

            loop_cm = (
                tc.For_i(0, loop_reps, 1)
                if loop_reps > 1
                else contextlib.nullcontext()
            )
            with loop_cm:
                # g outer / m inner: once f group g lands in SBUF there are 32
                # row-tiles (~60us) of work against it, so compute never
                # starves on the f DMA stream.
                for g in range(NG):
                    for m in range(M_TILES):
                        pt = psum_pool.tile([P, PW], f32, name="pt")
                        if fp8:
                            for j in range(JPG):
                                n0 = g * PW + j * MM_N
                                nc.tensor.matmul(
                                    pt[:, j * MM_N : (j + 1) * MM_N],
                                    lhsT=x_sb[m][:],
                                    rhs=f_sb[n0 // FW][:, :, n0 % FW : n0 % FW + MM_N],
                                    start=True,
                                    stop=True,
                                    perf_mode=mybir.MatmulPerfMode.DoubleRow,
                                )
                        else:
                            for k in range(KC):
                                for j in range(JPG):
                                    n0 = g * PW + j * MM_N
                                    nc.tensor.matmul(
                                        pt[:, j * MM_N : (j + 1) * MM_N],
                                        lhsT=x_sb[k][m][:],
                                        rhs=f_sb[k][n0 // FW][:, n0 % FW : n0 % FW + MM_N],
                                        start=(k == 0),
                                        stop=(k == KC - 1),
                                    )
                        col = m * NG + g
                        if variant == "act_accum":
                            ex = scratch.tile([P, PW], bf16, name="ex")
                            nc.scalar.activation(
                                ex[:],
                                pt[:],
                                mybir.ActivationFunctionType.Exp,
                                bias=bias_t[:, 0:1],
                                scale=act_scale,
                                accum_out=acc_sb[:, col : col + 1],
                            )
                        elif variant == "act_accum_inplace":
                            nc.scalar.activation(
                                pt[:],
                                pt[:],
                                mybir.ActivationFunctionType.Exp,
                                bias=bias_t[:, 0:1],
                                scale=act_scale,
                                accum_out=acc_sb[:, col : col + 1],
                            )
                        elif variant in ("dve_ts_accum", "fp8_dr"):
                            ex = scratch.tile([P, PW], bf16, name="ex")
                            junk = scratch.tile([P, PW], bf16, name="junk")
                            nc.scalar.activation(
                                ex[:],
                                pt[:],
                                mybir.ActivationFunctionType.Exp,
                                bias=bias_t[:, 0:1],
                                scale=act_scale,
                            )
                            nc.vector.tensor_scalar(
                                junk[:],
                                ex[:],
                                1.0,
                                0.0,
                                mybir.AluOpType.mult,
                                mybir.AluOpType.add,
                                accum_out=acc_sb[:, col : col + 1],
                            )
                        elif variant == "dve_reduce":
                            ex = scratch.tile([P, PW], bf16, name="ex")
                            nc.scalar.activation(
                                ex[:],
                                pt[:],
                                mybir.ActivationFunctionType.Exp,
                                bias=bias_t[:, 0:1],
                                scale=act_scale,
                            )
                            nc.vector.tensor_reduce(
                                acc_sb[:, col : col + 1],
                                ex[:],
                                axis=mybir.AxisListType.X,
                                op=mybir.AluOpType.add,
                            )
                        else:
                            raise ValueError(variant)

            nc.sync.dma_start(s_out[:, :], acc_sb[:])

    nc.compile()
    return nc


VARIANT = "dve_ts_accum"  # fp8_dr measured identical on HW; bf16 is more accurate


def _prep_in_maps(x, features, variant):
    if variant == "fp8_dr":
        xq = (x * FP8_SCALE).astype(ml_dtypes.float8_e4m3)
        xT = np.ascontiguousarray(xq.T.reshape(KC, P, B).transpose(1, 0, 2))
        fq = (features * FP8_SCALE).astype(ml_dtypes.float8_e4m3)
        in_maps = []
        for c in range(NCORES):
            shard = fq[c * NSH : (c + 1) * NSH]
            fT = np.ascontiguousarray(
                shard.T.reshape(KC, P, NSH).transpose(1, 0, 2)
            )
            in_maps.append({"xT": xT, "fT": fT})
    else:
        xb = x.astype(ml_dtypes.bfloat16)
        xT = np.ascontiguousarray(xb.T).reshape(KC, P, B)
        fb = features.astype(ml_dtypes.bfloat16)
        in_maps = []
        for c in range(NCORES):
            shard = fb[c * NSH : (c + 1) * NSH]      # [NSH, D]
            fT = np.ascontiguousarray(shard.T).reshape(KC, P, NSH)
            in_maps.append({"xT": xT, "fT": fT})
    return in_maps


def kernel(inputs, targets, feature_memory, k, features):
    from concourse.bass_utils import run_bass_kernel_spmd

    inputs = np.asarray(inputs, dtype=np.float32)
    targets = np.asarray(targets).astype(np.int64)
    features = np.asarray(features, dtype=np.float32)

    # Host prep: normalize (fp32, matching the reference), quantize,
    # pre-transpose to the PE-friendly layouts.
    nrm = np.linalg.norm(inputs, axis=1, keepdims=True)
    x = inputs / np.maximum(nrm, EPS)
    in_maps = _prep_in_maps(x, features, VARIANT)

    # Exact target logits on host (tiny: B dot-products).
    f_t = features[targets].astype(np.float64)
    tgt = SHIFT * np.einsum("bd,bd->b", x.astype(np.float64), f_t)

    if "nc" not in _CACHE:
        _CACHE["nc"] = _build_module(variant=VARIANT)
    nc = _CACHE["nc"]

    res = run_bass_kernel_spmd(nc, in_maps, core_ids=list(range(NCORES)))

    # Unshard: S_total[b] = sum over cores/groups; b = m*128 + p.
    S = np.zeros(B, dtype=np.float64)
    for c in range(NCORES):
        R = res.results[c]["s_out"].astype(np.float64)        # [P, M_TILES*NG]
        S += R.reshape(P, M_TILES, NG).sum(axis=2).T.reshape(B)

    lse = SHIFT + np.log(S)
    loss = np.mean(lse - tgt)
    return np.array(loss, dtype=np.float32)
